# revision 14
# baseline (speedup 1.0000x reference)
"""BERT-style dense transformer on 8 Trainium2 NeuronCores (one SPMD launch).

Sharding: token-parallel everywhere (256 tokens/core). The only collective is
a single pair AllGather of K/V for attention (cores 2b/2b+1 share batch b),
whose latency hides behind the Q projection. The vocab projection streams the
full [768, 32000] weight per core; log_softmax is computed fully locally.

All LN gains/biases are folded into the adjacent weights host-side, so the
device only computes the pure normalize (x - mean) * rsqrt(var + eps).
"""
import numpy as np
import ml_dtypes

import concourse.bass as bass
import concourse.tile as tile
from concourse import bacc, mybir
from concourse.bass_utils import run_bass_kernel_spmd
from concourse.masks import make_identity

F32 = mybir.dt.float32
BF16 = mybir.dt.bfloat16
I32 = mybir.dt.int32
AFT = mybir.ActivationFunctionType
ALU = mybir.AluOpType
AX = mybir.AxisListType

V, D, E, H, B, S = 32000, 768, 768, 12, 4, 512
EPS = 1e-5
NC = 8
TPC = (B * S) // NC          # 256 tokens per core
NT = B * S                   # 2048 total tokens
RG_PAIR = [[0, 1], [2, 3], [4, 5], [6, 7]]
KVHALF = H * E * TPC         # elements in one K^T (or V) block

BF = ml_dtypes.bfloat16


def _layernorm_rows(nc, sm, eps_tile, x):
    """In-place row LN (no gain/bias) on x [128, 768] f32 in SBUF."""
    stats = sm.tile([128, 3, 6], F32, tag="lnstats", name="lnstats")
    xg = x.rearrange("p (n f) -> p n f", f=256)
    for i in range(3):
        nc.vector.bn_stats(out=stats[:, i, :], in_=xg[:, i, :])
    mv = sm.tile([128, 2], F32, tag="lnmv", name="lnmv")
    nc.vector.bn_aggr(out=mv[:], in_=stats[:])
    nc.scalar.activation(out=mv[:, 1:2], in_=mv[:, 1:2], func=AFT.Sqrt,
                         bias=eps_tile[:], scale=1.0)
    nc.vector.reciprocal(out=mv[:, 1:2], in_=mv[:, 1:2])
    nc.vector.tensor_scalar(out=x[:], in0=x[:], scalar1=mv[:, 0:1],
                            scalar2=mv[:, 1:2], op0=ALU.subtract, op1=ALU.mult)


def build_nc(sim_gelu=False, skip=frozenset()):
    nc = bacc.Bacc("TRN2", target_bir_lowering=False, debug=False,
                   num_devices=NC)

    # ---- DRAM I/O ----
    ids_d = nc.dram_tensor("ids", [2, 128, 1], I32, kind="ExternalInput")
    static_d = nc.dram_tensor("static", [2, 128, D], F32, kind="ExternalInput")
    temb_d = nc.dram_tensor("tok_emb", [V, D], F32, kind="ExternalInput")
    wqkv_d = nc.dram_tensor("wqkv", [D, 3 * H * E], BF16, kind="ExternalInput")
    bqk_d = nc.dram_tensor("bqk", [128, 144], F32, kind="ExternalInput")
    bv_d = nc.dram_tensor("bv", [1, H * E], BF16, kind="ExternalInput")
    wo_d = nc.dram_tensor("wo", [H * E, D], BF16, kind="ExternalInput")
    bo_d = nc.dram_tensor("bo", [1, D], BF16, kind="ExternalInput")
    w1_d = nc.dram_tensor("w1", [D, E], BF16, kind="ExternalInput")
    b1p_d = nc.dram_tensor("b1p", [128, 6], F32, kind="ExternalInput")
    w2_d = nc.dram_tensor("w2", [E, D], BF16, kind="ExternalInput")
    b2_d = nc.dram_tensor("b2", [1, D], BF16, kind="ExternalInput")
    wp_d = nc.dram_tensor("wp", [D, V], BF16, kind="ExternalInput")
    bp_d = nc.dram_tensor("bp", [1, V], BF16, kind="ExternalInput")
    wc_d = nc.dram_tensor("wc", [D, 2], F32, kind="ExternalInput")
    bc_d = nc.dram_tensor("bc", [2, 1], F32, kind="ExternalInput")

    out_logits = nc.dram_tensor("out_logits", [TPC, V], F32,
                                kind="ExternalOutput")
    out_cls = nc.dram_tensor("out_cls", [2, 1], F32, kind="ExternalOutput")

    with tile.TileContext(nc) as tc:
        _emit(tc, nc, sim_gelu, skip,
              ids_d, static_d, temb_d, wqkv_d, bqk_d, bv_d, wo_d, bo_d,
              w1_d, b1p_d, w2_d, b2_d, wp_d, bp_d, wc_d, bc_d,
              out_logits, out_cls)
    nc.compile()
    return nc


def _emit(tc, nc, sim_gelu, skip,
          ids_d, static_d, temb_d, wqkv_d, bqk_d, bv_d, wo_d, bo_d,
          w1_d, b1p_d, w2_d, b2_d, wp_d, bp_d, wc_d, bc_d,
          out_logits, out_cls):
    gelu_f = AFT.Identity if sim_gelu else AFT.Gelu

    with tc.tile_pool(name="const", bufs=1) as cp, \
         tc.tile_pool(name="small", bufs=4) as sm, \
         tc.tile_pool(name="ps", bufs=3, space="PSUM") as ps, \
         tc.tile_pool(name="dram", bufs=1, space="DRAM") as dram:

        # ---- constants ----
        ident_f = cp.tile([128, 128], F32)
        make_identity(nc, ident_f)
        ident_b = cp.tile([128, 128], BF16)
        make_identity(nc, ident_b)
        ones_b = cp.tile([1, 128], BF16)
        nc.vector.memset(ones_b[:], 1.0)
        eps_t = cp.tile([128, 1], F32)
        nc.vector.memset(eps_t[:], EPS)
        bqk_sb = cp.tile([128, 144], F32)
        nc.sync.dma_start(out=bqk_sb[:], in_=bqk_d[:])
        b1p_sb = cp.tile([128, 6], F32)
        nc.sync.dma_start(out=b1p_sb[:], in_=b1p_d[:])
        wc_sb = cp.tile([128, 6, 2], F32)
        nc.sync.dma_start(out=wc_sb[:], in_=wc_d[:].rearrange(
            "(s p) n -> p s n", p=128))
        bc_sb = cp.tile([2, 1], F32)
        nc.sync.dma_start(out=bc_sb[:], in_=bc_d[:])

        # ---- DRAM bounce: merged K^T|V buffer, one pair AllGather ----
        kv_loc = dram.tile([2, KVHALF], BF16)
        kv_full = dram.tile([2, 2, KVHALF], BF16)
        kT_view = kv_loc[0, :].rearrange("(m p t) -> p m t", p=128, t=TPC)
        v_view = kv_loc[1, :].rearrange("(j p e) -> p j e", p=128, e=H * E)

        with tc.tile_pool(name="pEF", bufs=1) as pEF:
            et = pEF.tile([128, 6, TPC], BF16)       # final encoding, E-major
            encT0 = pEF.tile([128, 6], F32)          # token-0 column, f32

            with tc.tile_pool(name="mid", bufs=1) as mid:
                qT = mid.tile([128, 72, TPC], BF16)
                ctxT = mid.tile([128, 72, TPC], BF16)

                # ============ Phase A: embedding + LN + transpose ========
                with tc.tile_pool(name="pA", bufs=1) as pA, \
                     tc.tile_pool(name="wstream", bufs=3) as ws:
                    xT = pA.tile([128, 6, TPC], BF16)
                    for j in range(2):
                        ids_sb = sm.tile([128, 1], I32, tag="ids",
                                         name="ids_sb")
                        nc.sync.dma_start(out=ids_sb[:], in_=ids_d[j])
                        xj = pA.tile([128, D], F32, tag="xj", bufs=2,
                                     name="xj")
                        nc.gpsimd.indirect_dma_start(
                            out=xj[:], out_offset=None, in_=temb_d[:],
                            in_offset=bass.IndirectOffsetOnAxis(
                                ap=ids_sb[:, 0:1], axis=0))
                        st = pA.tile([128, D], F32, tag="st", bufs=2,
                                     name="st")
                        nc.sync.dma_start(out=st[:], in_=static_d[j])
                        nc.vector.tensor_add(out=xj[:], in0=xj[:], in1=st[:])
                        _layernorm_rows(nc, sm, eps_t, xj[:])
                        for s in range(6):
                            pt = ps.tile([128, 512], F32, tag="t", name="pt")
                            nc.tensor.transpose(
                                pt[:, :128], xj[:, s * 128:(s + 1) * 128],
                                ident_f[:])
                            nc.vector.tensor_copy(
                                out=xT[:, s, j * 128:(j + 1) * 128],
                                in_=pt[:, :128])

                    # ========= Phase B: QKV (K,V first; Q overlaps CC) ====
                    for c in list(range(18, 54)) + list(range(18)):
                        wch = ws.tile([128, 6, 512], BF16, tag="wch",
                                      name="wch")
                        nc.sync.dma_start(
                            out=wch[:],
                            in_=wqkv_d[:, c * 512:(c + 1) * 512].rearrange(
                                "(s p) n -> p s n", p=128))
                        if c < 36:
                            # Q (c<18) / K (18<=c<36): e-major output
                            for msub in range(4):
                                mg = c * 4 + msub
                                pq = ps.tile([128, 512], F32, tag="t",
                                             name="pq")
                                for s in range(6):
                                    nc.tensor.matmul(
                                        pq[:, :TPC],
                                        lhsT=wch[:, s,
                                                 msub * 128:(msub + 1) * 128],
                                        rhs=xT[:, s, :],
                                        start=(s == 0), stop=(s == 5))
                                dst = (qT[:, mg, :] if mg < 72 else
                                       ws.tile([128, TPC], BF16, tag="kout",
                                               name="kout")[:])
                                if "bqk" in skip:
                                    nc.vector.tensor_copy(out=dst,
                                                          in_=pq[:, :TPC])
                                else:
                                    nc.scalar.activation(
                                        out=dst, in_=pq[:, :TPC],
                                        func=AFT.Identity,
                                        bias=bqk_sb[:, mg:mg + 1], scale=1.0)
                                if mg >= 72:
                                    nc.sync.dma_start(
                                        out=kT_view[:, mg - 72, :], in_=dst)
                        else:
                            cv = c - 36
                            nobias = "bv" in skip
                            if not nobias:
                                bvs = ws.tile([1, 512], BF16, tag="bvs",
                                              name="bvs")
                                nc.sync.dma_start(
                                    out=bvs[:],
                                    in_=bv_d[:, cv * 512:(cv + 1) * 512])
                            for j in range(2):
                                pv = ps.tile([128, 512], F32, tag="t",
                                             name="pv")
                                for s in range(6):
                                    nc.tensor.matmul(
                                        pv[:],
                                        lhsT=xT[:, s, j * 128:(j + 1) * 128],
                                        rhs=wch[:, s, :],
                                        start=(s == 0),
                                        stop=(s == 5 and nobias))
                                if not nobias:
                                    nc.tensor.matmul(
                                        pv[:], lhsT=ones_b[:], rhs=bvs[:],
                                        start=False, stop=True)
                                vout = ws.tile([128, 512], BF16, tag="vout",
                                               name="vout")
                                nc.vector.tensor_copy(out=vout[:], in_=pv[:])
                                nc.sync.dma_start(
                                    out=v_view[:, j,
                                               cv * 512:(cv + 1) * 512],
                                    in_=vout[:])
                        if c == 53:
                            # K and V fully written -> gather while Q runs
                            nc.gpsimd.collective_compute(
                                "AllGather", ALU.bypass,
                                replica_groups=RG_PAIR,
                                ins=[kv_loc[:].opt()],
                                outs=[kv_full[:].opt()])

                # ================= Phase C: attention ====================
                with tc.tile_pool(name="pC", bufs=3) as pC:
                    for h in range(12):
                        kt = pC.tile([128, 6, 2 * TPC], BF16, tag="kt",
                                     name="kt")
                        vt = pC.tile([128, 4, E], BF16, tag="vt", name="vt")
                        for blk in range(2):
                            ktv = kv_full[blk, 0, :].rearrange(
                                "(m p t) -> p m t", p=128, t=TPC)
                            vtv = kv_full[blk, 1, :].rearrange(
                                "(j p e) -> p j e", p=128, e=H * E)
                            nc.sync.dma_start(
                                out=kt[:, :, blk * TPC:(blk + 1) * TPC],
                                in_=ktv[:, h * 6:(h + 1) * 6, :])
                            nc.sync.dma_start(
                                out=vt[:, blk * 2:blk * 2 + 2, :],
                                in_=vtv[:, :, h * E:(h + 1) * E])
                        probsT = pC.tile([128, 4, TPC], BF16, tag="probsT",
                                         name="probsT")
                        for j in range(2):
                            ssc = ps.tile([128, 512], F32, tag="t",
                                          name="ssc")
                            for s in range(6):
                                nc.tensor.matmul(
                                    ssc[:],
                                    lhsT=qT[:, h * 6 + s,
                                            j * 128:(j + 1) * 128],
                                    rhs=kt[:, s, :],
                                    start=(s == 0), stop=(s == 5))
                            negmax = sm.tile([128, 1], F32, tag="negmax",
                                             name="negmax")
                            nc.vector.tensor_reduce(
                                out=negmax[:], in_=ssc[:], axis=AX.X,
                                op=ALU.max, negate=True)
                            probs_f = pC.tile([128, 512], F32, tag="probs_f",
                                              name="probs_f")
                            sumexp = sm.tile([128, 1], F32, tag="sumexp",
                                             name="sumexp")
                            nc.scalar.activation(
                                out=probs_f[:], in_=ssc[:], func=AFT.Exp,
                                bias=negmax[:], scale=1.0,
                                accum_out=sumexp[:])
                            nc.vector.reciprocal(out=sumexp[:],
                                                 in_=sumexp[:])
                            probs_b = pC.tile([128, 512], BF16,
                                              tag="probs_b", name="probs_b")
                            nc.vector.tensor_scalar_mul(
                                out=probs_b[:], in0=probs_f[:],
                                scalar1=sumexp[:])
                            for kk in range(4):
                                ptr = ps.tile([128, 512], BF16, tag="t",
                                              name="ptr")
                                nc.tensor.transpose(
                                    ptr[:, :128],
                                    probs_b[:, kk * 128:(kk + 1) * 128],
                                    ident_b[:])
                                nc.vector.tensor_copy(
                                    out=probsT[:, kk, j * 128:(j + 1) * 128],
                                    in_=ptr[:, :128])
                        for m in range(6):
                            pc_ = ps.tile([128, 512], F32, tag="t",
                                          name="pc_")
                            for jj in range(4):
                                nc.tensor.matmul(
                                    pc_[:, :TPC],
                                    lhsT=vt[:, jj, m * 128:(m + 1) * 128],
                                    rhs=probsT[:, jj, :],
                                    start=(jj == 0), stop=(jj == 3))
                            nc.scalar.activation(
                                out=ctxT[:, h * 6 + m, :], in_=pc_[:, :TPC],
                                func=AFT.Copy)

                # ============== Phase D: Wo proj + LN ====================
                with tc.tile_pool(name="pD", bufs=1) as pD, \
                     tc.tile_pool(name="wsD", bufs=2) as wsD, \
                     tc.tile_pool(name="psD", bufs=1, space="PSUM") as psD:
                    bo_sb = b2_sb = None
                    if "bo" not in skip:
                        bo_sb = pD.tile([1, D], BF16)
                        nc.sync.dma_start(out=bo_sb[:], in_=bo_d[:])
                    if "b2" not in skip:
                        b2_sb = pD.tile([1, D], BF16)
                        nc.sync.dma_start(out=b2_sb[:], in_=b2_d[:])
                    po = [[psD.tile([128, 512], F32, tag=f"po{j}{n}",
                                    name=f"po{j}{n}")
                           for n in range(2)] for j in range(2)]
                    for kc in range(6):
                        woch = wsD.tile([128, 12, D], BF16, tag="woch",
                                        name="woch")
                        nc.sync.dma_start(
                            out=woch[:],
                            in_=wo_d[kc * 1536:(kc + 1) * 1536, :].rearrange(
                                "(k p) n -> p k n", p=128))
                        for j in range(2):
                            for ki in range(12):
                                kg = kc * 12 + ki
                                last = (kg == 71) and ("bo" in skip)
                                nc.tensor.matmul(
                                    po[j][0][:],
                                    lhsT=ctxT[:, kg, j * 128:(j + 1) * 128],
                                    rhs=woch[:, ki, 0:512],
                                    start=(kg == 0), stop=last)
                                nc.tensor.matmul(
                                    po[j][1][:, :256],
                                    lhsT=ctxT[:, kg, j * 128:(j + 1) * 128],
                                    rhs=woch[:, ki, 512:768],
                                    start=(kg == 0), stop=last)
                    ar = pD.tile([128, 2, D], F32)
                    for j in range(2):
                        if "bo" not in skip:
                            nc.tensor.matmul(po[j][0][:], lhsT=ones_b[:],
                                             rhs=bo_sb[:, 0:512],
                                             start=False, stop=True)
                            nc.tensor.matmul(po[j][1][:, :256],
                                             lhsT=ones_b[:],
                                             rhs=bo_sb[:, 512:768],
                                             start=False, stop=True)
                        nc.vector.tensor_copy(out=ar[:, j, 0:512],
                                              in_=po[j][0][:])
                        nc.vector.tensor_copy(out=ar[:, j, 512:768],
                                              in_=po[j][1][:, :256])
                    aT = pD.tile([128, 6, TPC], BF16)
                    for j in range(2):
                        _layernorm_rows(nc, sm, eps_t, ar[:, j, :])
                        for s in range(6):
                            pt2 = ps.tile([128, 512], F32, tag="t",
                                          name="pt2")
                            nc.tensor.transpose(
                                pt2[:, :128], ar[:, j, s * 128:(s + 1) * 128],
                                ident_f[:])
                            nc.vector.tensor_copy(
                                out=aT[:, s, j * 128:(j + 1) * 128],
                                in_=pt2[:, :128])

                    # ========== Phase E: FFN + LN + cls ==================
                    w1ch = pD.tile([128, 6, E], BF16)
                    nc.sync.dma_start(
                        out=w1ch[:],
                        in_=w1_d[:].rearrange("(s p) n -> p s n", p=128))
                    hT = pD.tile([128, 6, TPC], BF16)
                    for m in range(6):
                        ph = ps.tile([128, 512], F32, tag="t", name="ph")
                        for s in range(6):
                            nc.tensor.matmul(
                                ph[:, :TPC],
                                lhsT=w1ch[:, s, m * 128:(m + 1) * 128],
                                rhs=aT[:, s, :], start=(s == 0), stop=(s == 5))
                        nc.scalar.activation(
                            out=hT[:, m, :], in_=ph[:, :TPC], func=gelu_f,
                            bias=b1p_sb[:, m:m + 1], scale=1.0)
                    w2ch = pD.tile([128, 6, D], BF16)
                    nc.sync.dma_start(
                        out=w2ch[:],
                        in_=w2_d[:].rearrange("(s p) n -> p s n", p=128))
                    pe = [[psD.tile([128, 512], F32, tag=f"po{j}{n}",
                                    name=f"pe{j}{n}")
                           for n in range(2)] for j in range(2)]
                    er = pD.tile([128, 2, D], F32)
                    for j in range(2):
                        for s in range(6):
                            last = (s == 5) and ("b2" in skip)
                            nc.tensor.matmul(
                                pe[j][0][:],
                                lhsT=hT[:, s, j * 128:(j + 1) * 128],
                                rhs=w2ch[:, s, 0:512], start=(s == 0),
                                stop=last)
                            nc.tensor.matmul(
                                pe[j][1][:, :256],
                                lhsT=hT[:, s, j * 128:(j + 1) * 128],
                                rhs=w2ch[:, s, 512:768], start=(s == 0),
                                stop=last)
                        if "b2" not in skip:
                            nc.tensor.matmul(pe[j][0][:], lhsT=ones_b[:],
                                             rhs=b2_sb[:, 0:512],
                                             start=False, stop=True)
                            nc.tensor.matmul(pe[j][1][:, :256],
                                             lhsT=ones_b[:],
                                             rhs=b2_sb[:, 512:768],
                                             start=False, stop=True)
                        nc.vector.tensor_copy(out=er[:, j, 0:512],
                                              in_=pe[j][0][:])
                        nc.vector.tensor_copy(out=er[:, j, 512:768],
                                              in_=pe[j][1][:, :256])
                    for j in range(2):
                        _layernorm_rows(nc, sm, eps_t, er[:, j, :])
                        for s in range(6):
                            pt3 = ps.tile([128, 512], F32, tag="t",
                                          name="pt3")
                            nc.tensor.transpose(
                                pt3[:, :128], er[:, j, s * 128:(s + 1) * 128],
                                ident_f[:])
                            nc.vector.tensor_copy(
                                out=et[:, s, j * 128:(j + 1) * 128],
                                in_=pt3[:, :128])
                            if j == 0:
                                nc.scalar.activation(out=encT0[:, s:s + 1],
                                                     in_=pt3[:, 0:1],
                                                     func=AFT.Copy)
                    # cls head (f32 exact)
                    with tc.tile_pool(name="pscls", bufs=1,
                                      space="PSUM") as pscls:
                        pcl = pscls.tile([2, 512], F32)
                        for s in range(6):
                            nc.tensor.matmul(pcl[:, 0:1], lhsT=wc_sb[:, s, :],
                                             rhs=encT0[:, s:s + 1],
                                             start=(s == 0), stop=(s == 5))
                        cls_sb = sm.tile([2, 1], F32, tag="cls",
                                         name="cls_sb")
                        nc.scalar.activation(out=cls_sb[:], in_=pcl[:, 0:1],
                                             func=AFT.Identity, bias=bc_sb[:],
                                             scale=1.0)
                        nc.sync.dma_start(out=out_cls[:], in_=cls_sb[:])
            # mid pool (qT/ctxT) closed here

            # ============ Phase F: vocab projection (fully local) ========
            # per token-tile j: 64 chunks of 500 vocab cols; local logsumexp.
            NCH = 64
            CW = 500
            with tc.tile_pool(name="pF", bufs=1) as pF, \
                 tc.tile_pool(name="wsF", bufs=3) as wsF, \
                 tc.tile_pool(name="ltp", bufs=2) as ltp, \
                 tc.tile_pool(name="statp", bufs=2) as stp, \
                 tc.tile_pool(name="scrp", bufs=3) as scrp:
                bp_sb = None
                if "bp" not in skip:
                    bp_sb = pF.tile([1, V], BF16)
                    nc.sync.dma_start(out=bp_sb[:], in_=bp_d[:])
                for j in range(2):
                    lt = ltp.tile([128, V], BF16, tag="lt", name=f"lt{j}")
                    nm = stp.tile([128, NCH], F32, tag="nm", name=f"nm{j}")
                    se = stp.tile([128, 16], F32, tag="se", name=f"se{j}")
                    for vc in range(NCH):
                        wpch = wsF.tile([128, 6, CW], BF16, tag="wpch",
                                        name="wpch")
                        nc.sync.dma_start(
                            out=wpch[:],
                            in_=wp_d[:, vc * CW:(vc + 1) * CW].rearrange(
                                "(s p) v -> p s v", p=128))
                        pl = ps.tile([128, 512], F32, tag="t", name="pl")
                        for s in range(6):
                            last = (s == 5) and ("bp" in skip)
                            nc.tensor.matmul(
                                pl[:, :CW],
                                lhsT=et[:, s, j * 128:(j + 1) * 128],
                                rhs=wpch[:, s, :],
                                start=(s == 0), stop=last)
                        if "bp" not in skip:
                            nc.tensor.matmul(
                                pl[:, :CW], lhsT=ones_b[:],
                                rhs=bp_sb[:, vc * CW:(vc + 1) * CW],
                                start=False, stop=True)
                        if vc % 2 == 0:
                            nc.vector.tensor_copy(
                                out=lt[:, vc * CW:(vc + 1) * CW],
                                in_=pl[:, :CW])
                        else:
                            nc.scalar.activation(
                                out=lt[:, vc * CW:(vc + 1) * CW],
                                in_=pl[:, :CW], func=AFT.Copy)
                        nc.vector.tensor_reduce(
                            out=nm[:, vc:vc + 1], in_=pl[:, :CW], axis=AX.X,
                            op=ALU.max, negate=True)
                    gneg = stp.tile([128, 1], F32, tag="gneg", name=f"gn{j}")
                    nc.vector.tensor_reduce(out=gneg[:], in_=nm[:],
                                            axis=AX.X, op=ALU.min)
                    # sum of exp(x - M) per 2000-wide block, ACT accum
                    for bb in range(16):
                        scr = scrp.tile([128, 2000], BF16, tag="scr",
                                        name="scr")
                        nc.scalar.activation(
                            out=scr[:], in_=lt[:, bb * 2000:(bb + 1) * 2000],
                            func=AFT.Exp, bias=gneg[:], scale=1.0,
                            accum_out=se[:, bb:bb + 1])
                    ssum = stp.tile([128, 1], F32, tag="ssum", name=f"ss{j}")
                    nc.vector.tensor_reduce(out=ssum[:], in_=se[:],
                                            axis=AX.X, op=ALU.add)
                    nc.scalar.activation(out=ssum[:], in_=ssum[:],
                                         func=AFT.Ln)
                    lse_neg = stp.tile([128, 1], F32, tag="lse",
                                       name=f"lse{j}")
                    nc.vector.tensor_sub(out=lse_neg[:], in0=gneg[:],
                                         in1=ssum[:])
                    for ob in range(16):
                        obuf = scrp.tile([128, 2000], F32, tag="obuf",
                                         name="obuf")
                        if ob % 2 == 0:
                            nc.scalar.activation(
                                out=obuf[:],
                                in_=lt[:, ob * 2000:(ob + 1) * 2000],
                                func=AFT.Identity, bias=lse_neg[:],
                                scale=1.0)
                        else:
                            nc.vector.tensor_scalar_add(
                                out=obuf[:],
                                in0=lt[:, ob * 2000:(ob + 1) * 2000],
                                scalar1=lse_neg[:])
                        nc.sync.dma_start(
                            out=out_logits[j * 128:(j + 1) * 128,
                                           ob * 2000:(ob + 1) * 2000],
                            in_=obuf[:])


# ======================= host side ==================================

def _host_prep(inputs):
    g = lambda k: np.asarray(inputs[k], dtype=np.float32)
    ids_full = np.asarray(inputs["input_tensor"]).astype(np.int32).reshape(-1)

    tok_emb = g("tok_emb")
    seg_emb = g("seg_emb")
    ln_eg, ln_eb = g("ln_emb_g"), g("ln_emb_b")
    Wq, bq = g("Wq"), g("bq")
    Wk, bk = g("Wk"), g("bk")
    Wv, bv = g("Wv"), g("bv")
    Wo, bo = g("Wo"), g("bo")
    ln_ag, ln_ab = g("ln_attn_g"), g("ln_attn_b")
    W1, b1 = g("W1"), g("b1")
    W2, b2 = g("W2"), g("b2")
    ln_fg, ln_fb = g("ln_ff_g"), g("ln_ff_b")
    Wp, bp = g("Wp"), g("bp")
    Wc, bc = g("Wc"), g("bc")

    # positional + segment "static" add, per sequence position
    p = np.arange(S, dtype=np.float32)[:, None]
    d2 = 2.0 * np.arange(D, dtype=np.float32) / D
    ang = p / np.power(np.float32(10000.0), d2)
    even = (np.arange(D) % 2) == 0
    pos = np.where(even[None, :], np.sin(ang), np.cos(ang)).astype(np.float32)
    seg_ids = np.where(np.arange(S) >= S // 2 + 1, 1, 0)
    static = (seg_emb[seg_ids] + pos).astype(np.float32)        # [S, D]
    static_full = np.tile(static, (B, 1))                        # [NT, D]

    scale = 1.0 / np.sqrt(np.float32(S))
    Wq_t = Wq.transpose(1, 0, 2).reshape(D, H * E)
    Wk_t = Wk.transpose(1, 0, 2).reshape(D, H * E)
    Wv_t = Wv.transpose(1, 0, 2).reshape(D, H * E)
    wq_f = (ln_eg[:, None] * Wq_t) * scale
    wk_f = ln_eg[:, None] * Wk_t
    wv_f = ln_eg[:, None] * Wv_t
    bq_f = (ln_eb @ Wq_t + bq.reshape(-1)) * scale
    bk_f = ln_eb @ Wk_t + bk.reshape(-1)
    bv_f = ln_eb @ Wv_t + bv.reshape(-1)
    wqkv = np.concatenate([wq_f, wk_f, wv_f], axis=1).astype(BF)
    bqk = np.concatenate([bq_f, bk_f]).reshape(144, 128).T
    bqk = np.ascontiguousarray(bqk, dtype=np.float32)
    bv_r = bv_f.reshape(1, H * E).astype(BF)

    wo_b = Wo.astype(BF)
    bo_r = bo.reshape(1, D).astype(BF)
    w1_f = (ln_ag[:, None] * W1).astype(BF)
    b1_f = (ln_ab @ W1 + b1).astype(np.float32)
    b1p = np.ascontiguousarray(b1_f.reshape(6, 128).T, dtype=np.float32)
    w2_b = W2.astype(BF)
    b2_r = b2.reshape(1, D).astype(BF)
    wp_f = (ln_fg[:, None] * Wp).astype(BF)
    bp_f = (ln_fb @ Wp + bp).astype(np.float32)
    wc_f = np.ascontiguousarray(ln_fg[:, None] * Wc, dtype=np.float32)
    bc_f = np.ascontiguousarray((ln_fb @ Wc + bc).reshape(2, 1),
                                dtype=np.float32)

    skip = set()
    if not np.any(np.concatenate([bq_f, bk_f])):
        skip.add("bqk")
    if not np.any(bv_f):
        skip.add("bv")
    if not np.any(bo):
        skip.add("bo")
    if not np.any(b2):
        skip.add("b2")
    if not np.any(bp_f):
        skip.add("bp")

    tok_emb_c = np.ascontiguousarray(tok_emb, dtype=np.float32)
    bp_r = np.ascontiguousarray(bp_f.reshape(1, V).astype(BF))
    in_maps = []
    for c in range(NC):
        sl = slice(c * TPC, (c + 1) * TPC)
        in_maps.append(dict(
            ids=np.ascontiguousarray(
                ids_full[sl].reshape(2, 128, 1), dtype=np.int32),
            static=np.ascontiguousarray(
                static_full[sl].reshape(2, 128, D), dtype=np.float32),
            tok_emb=tok_emb_c,
            wqkv=wqkv, bqk=bqk, bv=bv_r,
            wo=wo_b, bo=bo_r,
            w1=w1_f, b1p=b1p, w2=w2_b, b2=b2_r,
            wp=wp_f, bp=bp_r,
            wc=wc_f, bc=bc_f,
        ))
    return in_maps, frozenset(skip)


_NC_CACHE = {}


def _get_nc(sim_gelu=False, skip=frozenset()):
    key = (sim_gelu, skip)
    if key not in _NC_CACHE:
        _NC_CACHE[key] = build_nc(sim_gelu=sim_gelu, skip=skip)
    return _NC_CACHE[key]


def kernel(**inputs):
    in_maps, skip = _host_prep(inputs)
    nc = _get_nc(sim_gelu=False, skip=skip)
    res = run_bass_kernel_spmd(nc, in_maps, list(range(NC)))
    return _assemble(res.results)


def _assemble(results):
    token_pred = np.concatenate(
        [results[c]["out_logits"] for c in range(NC)], axis=0)
    token_pred = np.ascontiguousarray(
        token_pred.reshape(B, S, V), dtype=np.float32)
    cls = np.stack([results[2 * b]["out_cls"][:, 0] for b in range(B)])
    return token_pred, np.ascontiguousarray(cls, dtype=np.float32)


# revision 16
# speedup vs baseline: 1.4667x; 1.4667x over previous
"""BERT-style dense transformer on 8 Trainium2 NeuronCores (one SPMD launch).

Sharding: token-parallel everywhere (256 tokens/core). The only collective is
a single pair AllGather of K/V for attention (cores 2b/2b+1 share batch b),
whose latency hides behind the Q projection. The vocab projection streams the
full [768, 32000] weight per core; log_softmax is computed fully locally.

All LN gains/biases are folded into the adjacent weights host-side, so the
device only computes the pure normalize (x - mean) * rsqrt(var + eps).
"""
import numpy as np
import ml_dtypes

import concourse.bass as bass
import concourse.tile as tile
from concourse import bacc, mybir
from concourse.bass_utils import run_bass_kernel_spmd
from concourse.masks import make_identity

F32 = mybir.dt.float32
BF16 = mybir.dt.bfloat16
I32 = mybir.dt.int32
AFT = mybir.ActivationFunctionType
ALU = mybir.AluOpType
AX = mybir.AxisListType

V, D, E, H, B, S = 32000, 768, 768, 12, 4, 512
EPS = 1e-5
NC = 8
TPC = (B * S) // NC          # 256 tokens per core
NT = B * S                   # 2048 total tokens
RG_PAIR = [[0, 1], [2, 3], [4, 5], [6, 7]]
KVHALF = H * E * TPC         # elements in one K^T (or V) block

BF = ml_dtypes.bfloat16


def _layernorm_rows(nc, sm, eps_tile, x):
    """In-place row LN (no gain/bias) on x [128, 768] f32 in SBUF."""
    stats = sm.tile([128, 3, 6], F32, tag="lnstats", name="lnstats")
    xg = x.rearrange("p (n f) -> p n f", f=256)
    for i in range(3):
        nc.vector.bn_stats(out=stats[:, i, :], in_=xg[:, i, :])
    mv = sm.tile([128, 2], F32, tag="lnmv", name="lnmv")
    nc.vector.bn_aggr(out=mv[:], in_=stats[:])
    nc.scalar.activation(out=mv[:, 1:2], in_=mv[:, 1:2], func=AFT.Sqrt,
                         bias=eps_tile[:], scale=1.0)
    nc.vector.reciprocal(out=mv[:, 1:2], in_=mv[:, 1:2])
    nc.vector.tensor_scalar(out=x[:], in0=x[:], scalar1=mv[:, 0:1],
                            scalar2=mv[:, 1:2], op0=ALU.subtract, op1=ALU.mult)


def build_nc(sim_gelu=False, skip=frozenset()):
    nc = bacc.Bacc("TRN2", target_bir_lowering=False, debug=False,
                   num_devices=NC)

    # ---- DRAM I/O ----
    ids_d = nc.dram_tensor("ids", [4, 128, 1], I32, kind="ExternalInput")
    static_d = nc.dram_tensor("static", [4, 128, D], F32, kind="ExternalInput")
    temb_d = nc.dram_tensor("tok_emb", [V, D], F32, kind="ExternalInput")
    wqkv_d = nc.dram_tensor("wqkv", [D, 3 * H * E], BF16, kind="ExternalInput")
    bqk_d = nc.dram_tensor("bqk", [128, 144], F32, kind="ExternalInput")
    bv_d = nc.dram_tensor("bv", [1, H * E], BF16, kind="ExternalInput")
    wo_d = nc.dram_tensor("wo", [H * E, D], BF16, kind="ExternalInput")
    bo_d = nc.dram_tensor("bo", [1, D], BF16, kind="ExternalInput")
    w1_d = nc.dram_tensor("w1", [D, E], BF16, kind="ExternalInput")
    b1p_d = nc.dram_tensor("b1p", [128, 6], F32, kind="ExternalInput")
    w2_d = nc.dram_tensor("w2", [E, D], BF16, kind="ExternalInput")
    b2_d = nc.dram_tensor("b2", [1, D], BF16, kind="ExternalInput")
    wp_d = nc.dram_tensor("wp", [D, V], BF16, kind="ExternalInput")
    bp_d = nc.dram_tensor("bp", [1, V], BF16, kind="ExternalInput")
    wc_d = nc.dram_tensor("wc", [D, 2], F32, kind="ExternalInput")
    bc_d = nc.dram_tensor("bc", [2, 1], F32, kind="ExternalInput")

    out_logits = nc.dram_tensor("out_logits", [TPC, V], F32,
                                kind="ExternalOutput")
    out_cls = nc.dram_tensor("out_cls", [2, 1], F32, kind="ExternalOutput")

    with tile.TileContext(nc) as tc:
        _emit(tc, nc, sim_gelu, skip,
              ids_d, static_d, temb_d, wqkv_d, bqk_d, bv_d, wo_d, bo_d,
              w1_d, b1p_d, w2_d, b2_d, wp_d, bp_d, wc_d, bc_d,
              out_logits, out_cls)
    nc.compile()
    return nc


def _emit(tc, nc, sim_gelu, skip,
          ids_d, static_d, temb_d, wqkv_d, bqk_d, bv_d, wo_d, bo_d,
          w1_d, b1p_d, w2_d, b2_d, wp_d, bp_d, wc_d, bc_d,
          out_logits, out_cls):
    gelu_f = AFT.Identity if sim_gelu else AFT.Gelu

    with tc.tile_pool(name="const", bufs=1) as cp, \
         tc.tile_pool(name="small", bufs=4) as sm, \
         tc.tile_pool(name="ps", bufs=3, space="PSUM") as ps, \
         tc.tile_pool(name="dram", bufs=1, space="DRAM") as dram:

        # ---- constants ----
        ident_f = cp.tile([128, 128], F32)
        make_identity(nc, ident_f)
        ident_b = cp.tile([128, 128], BF16)
        make_identity(nc, ident_b)
        ones_b = cp.tile([1, 128], BF16)
        nc.vector.memset(ones_b[:], 1.0)
        eps_t = cp.tile([128, 1], F32)
        nc.vector.memset(eps_t[:], EPS)
        bqk_sb = cp.tile([128, 144], F32)
        nc.sync.dma_start(out=bqk_sb[:], in_=bqk_d[:])
        b1p_sb = cp.tile([128, 6], F32)
        nc.sync.dma_start(out=b1p_sb[:], in_=b1p_d[:])
        wc_sb = cp.tile([128, 6, 2], F32)
        nc.sync.dma_start(out=wc_sb[:], in_=wc_d[:].rearrange(
            "(s p) n -> p s n", p=128))
        bc_sb = cp.tile([2, 1], F32)
        nc.sync.dma_start(out=bc_sb[:], in_=bc_d[:])

        # ---- DRAM spill for full-batch K^T and V (no collectives) ----
        kT_loc = dram.tile([H * E, 2 * TPC], BF16)
        v_loc = dram.tile([2 * TPC, H * E], BF16)
        kT_view = kT_loc[:].rearrange("(m p) t -> p m t", p=128)
        v_view = v_loc[:].rearrange("(jt p) e -> p jt e", p=128)

        with tc.tile_pool(name="pEF", bufs=1) as pEF:
            et = pEF.tile([128, 6, TPC], BF16)       # final encoding, E-major
            encT0 = pEF.tile([128, 6], F32)          # token-0 column, f32

            with tc.tile_pool(name="mid", bufs=1) as mid:
                qT = mid.tile([128, 72, TPC], BF16)
                ctxT = mid.tile([128, 72, TPC], BF16)

                # ============ Phase A: embedding + LN + transpose ========
                with tc.tile_pool(name="pA", bufs=1) as pA, \
                     tc.tile_pool(name="wstream", bufs=3) as ws:
                    xT = pA.tile([128, 6, 2 * TPC], BF16)
                    for j in range(4):
                        ids_sb = sm.tile([128, 1], I32, tag="ids",
                                         name="ids_sb")
                        nc.sync.dma_start(out=ids_sb[:], in_=ids_d[j])
                        xj = pA.tile([128, D], F32, tag="xj", bufs=2,
                                     name="xj")
                        nc.gpsimd.indirect_dma_start(
                            out=xj[:], out_offset=None, in_=temb_d[:],
                            in_offset=bass.IndirectOffsetOnAxis(
                                ap=ids_sb[:, 0:1], axis=0))
                        st = pA.tile([128, D], F32, tag="st", bufs=2,
                                     name="st")
                        nc.sync.dma_start(out=st[:], in_=static_d[j])
                        nc.vector.tensor_add(out=xj[:], in0=xj[:], in1=st[:])
                        _layernorm_rows(nc, sm, eps_t, xj[:])
                        for s in range(6):
                            pt = ps.tile([128, 512], F32, tag="t", name="pt")
                            nc.tensor.transpose(
                                pt[:, :128], xj[:, s * 128:(s + 1) * 128],
                                ident_f[:])
                            nc.vector.tensor_copy(
                                out=xT[:, s, j * 128:(j + 1) * 128],
                                in_=pt[:, :128])

                    # ========= Phase B: QKV (K,V first; Q overlaps CC) ====
                    for c in list(range(18, 54)) + list(range(18)):
                        wch = ws.tile([128, 6, 512], BF16, tag="wch",
                                      name="wch")
                        nc.sync.dma_start(
                            out=wch[:],
                            in_=wqkv_d[:, c * 512:(c + 1) * 512].rearrange(
                                "(s p) n -> p s n", p=128))
                        if c < 36:
                            # Q (c<18, own half) / K (18<=c<36, full batch)
                            for msub in range(4):
                                mg = c * 4 + msub
                                wid = TPC if mg < 72 else 2 * TPC
                                pq = ps.tile([128, 512], F32, tag="t",
                                             name="pq")
                                for s in range(6):
                                    nc.tensor.matmul(
                                        pq[:, :wid],
                                        lhsT=wch[:, s,
                                                 msub * 128:(msub + 1) * 128],
                                        rhs=xT[:, s, :wid],
                                        start=(s == 0), stop=(s == 5))
                                dst = (qT[:, mg, :] if mg < 72 else
                                       ws.tile([128, 2 * TPC], BF16,
                                               tag="kout", name="kout")[:])
                                if "bqk" in skip:
                                    nc.vector.tensor_copy(out=dst,
                                                          in_=pq[:, :wid])
                                else:
                                    nc.scalar.activation(
                                        out=dst, in_=pq[:, :wid],
                                        func=AFT.Identity,
                                        bias=bqk_sb[:, mg:mg + 1], scale=1.0)
                                if mg >= 72:
                                    nc.sync.dma_start(
                                        out=kT_view[:, mg - 72, :], in_=dst)
                        else:
                            cv = c - 36
                            nobias = "bv" in skip
                            if not nobias:
                                bvs = ws.tile([1, 512], BF16, tag="bvs",
                                              name="bvs")
                                nc.sync.dma_start(
                                    out=bvs[:],
                                    in_=bv_d[:, cv * 512:(cv + 1) * 512])
                            for jt in range(4):
                                pv = ps.tile([128, 512], F32, tag="t",
                                             name="pv")
                                for s in range(6):
                                    nc.tensor.matmul(
                                        pv[:],
                                        lhsT=xT[:, s,
                                                jt * 128:(jt + 1) * 128],
                                        rhs=wch[:, s, :],
                                        start=(s == 0),
                                        stop=(s == 5 and nobias))
                                if not nobias:
                                    nc.tensor.matmul(
                                        pv[:], lhsT=ones_b[:], rhs=bvs[:],
                                        start=False, stop=True)
                                vout = ws.tile([128, 512], BF16, tag="vout",
                                               name="vout")
                                nc.vector.tensor_copy(out=vout[:], in_=pv[:])
                                nc.sync.dma_start(
                                    out=v_view[:, jt,
                                               cv * 512:(cv + 1) * 512],
                                    in_=vout[:])

                # ================= Phase C: attention ====================
                with tc.tile_pool(name="pC", bufs=3) as pC:
                    for h in range(12):
                        kt = pC.tile([128, 6, 2 * TPC], BF16, tag="kt",
                                     name="kt")
                        vt = pC.tile([128, 4, E], BF16, tag="vt", name="vt")
                        nc.sync.dma_start(
                            out=kt[:], in_=kT_view[:, h * 6:(h + 1) * 6, :])
                        nc.sync.dma_start(
                            out=vt[:], in_=v_view[:, :, h * E:(h + 1) * E])
                        probsT = pC.tile([128, 4, TPC], BF16, tag="probsT",
                                         name="probsT")
                        for j in range(2):
                            ssc = ps.tile([128, 512], F32, tag="t",
                                          name="ssc")
                            for s in range(6):
                                nc.tensor.matmul(
                                    ssc[:],
                                    lhsT=qT[:, h * 6 + s,
                                            j * 128:(j + 1) * 128],
                                    rhs=kt[:, s, :],
                                    start=(s == 0), stop=(s == 5))
                            negmax = sm.tile([128, 1], F32, tag="negmax",
                                             name="negmax")
                            nc.vector.tensor_reduce(
                                out=negmax[:], in_=ssc[:], axis=AX.X,
                                op=ALU.max, negate=True)
                            probs_f = pC.tile([128, 512], F32, tag="probs_f",
                                              name="probs_f")
                            sumexp = sm.tile([128, 1], F32, tag="sumexp",
                                             name="sumexp")
                            nc.scalar.activation(
                                out=probs_f[:], in_=ssc[:], func=AFT.Exp,
                                bias=negmax[:], scale=1.0,
                                accum_out=sumexp[:])
                            nc.vector.reciprocal(out=sumexp[:],
                                                 in_=sumexp[:])
                            probs_b = pC.tile([128, 512], BF16,
                                              tag="probs_b", name="probs_b")
                            nc.vector.tensor_scalar_mul(
                                out=probs_b[:], in0=probs_f[:],
                                scalar1=sumexp[:])
                            for kk in range(4):
                                ptr = ps.tile([128, 512], BF16, tag="t",
                                              name="ptr")
                                nc.tensor.transpose(
                                    ptr[:, :128],
                                    probs_b[:, kk * 128:(kk + 1) * 128],
                                    ident_b[:])
                                nc.vector.tensor_copy(
                                    out=probsT[:, kk, j * 128:(j + 1) * 128],
                                    in_=ptr[:, :128])
                        for m in range(6):
                            pc_ = ps.tile([128, 512], F32, tag="t",
                                          name="pc_")
                            for jj in range(4):
                                nc.tensor.matmul(
                                    pc_[:, :TPC],
                                    lhsT=vt[:, jj, m * 128:(m + 1) * 128],
                                    rhs=probsT[:, jj, :],
                                    start=(jj == 0), stop=(jj == 3))
                            nc.scalar.activation(
                                out=ctxT[:, h * 6 + m, :], in_=pc_[:, :TPC],
                                func=AFT.Copy)

                # ============== Phase D: Wo proj + LN ====================
                with tc.tile_pool(name="pD", bufs=1) as pD, \
                     tc.tile_pool(name="wsD", bufs=2) as wsD, \
                     tc.tile_pool(name="psD", bufs=1, space="PSUM") as psD:
                    bo_sb = b2_sb = None
                    if "bo" not in skip:
                        bo_sb = pD.tile([1, D], BF16)
                        nc.sync.dma_start(out=bo_sb[:], in_=bo_d[:])
                    if "b2" not in skip:
                        b2_sb = pD.tile([1, D], BF16)
                        nc.sync.dma_start(out=b2_sb[:], in_=b2_d[:])
                    po = [[psD.tile([128, 512], F32, tag=f"po{j}{n}",
                                    name=f"po{j}{n}")
                           for n in range(2)] for j in range(2)]
                    for kc in range(6):
                        woch = wsD.tile([128, 12, D], BF16, tag="woch",
                                        name="woch")
                        nc.sync.dma_start(
                            out=woch[:],
                            in_=wo_d[kc * 1536:(kc + 1) * 1536, :].rearrange(
                                "(k p) n -> p k n", p=128))
                        for j in range(2):
                            for ki in range(12):
                                kg = kc * 12 + ki
                                last = (kg == 71) and ("bo" in skip)
                                nc.tensor.matmul(
                                    po[j][0][:],
                                    lhsT=ctxT[:, kg, j * 128:(j + 1) * 128],
                                    rhs=woch[:, ki, 0:512],
                                    start=(kg == 0), stop=last)
                                nc.tensor.matmul(
                                    po[j][1][:, :256],
                                    lhsT=ctxT[:, kg, j * 128:(j + 1) * 128],
                                    rhs=woch[:, ki, 512:768],
                                    start=(kg == 0), stop=last)
                    ar = pD.tile([128, 2, D], F32)
                    for j in range(2):
                        if "bo" not in skip:
                            nc.tensor.matmul(po[j][0][:], lhsT=ones_b[:],
                                             rhs=bo_sb[:, 0:512],
                                             start=False, stop=True)
                            nc.tensor.matmul(po[j][1][:, :256],
                                             lhsT=ones_b[:],
                                             rhs=bo_sb[:, 512:768],
                                             start=False, stop=True)
                        nc.vector.tensor_copy(out=ar[:, j, 0:512],
                                              in_=po[j][0][:])
                        nc.vector.tensor_copy(out=ar[:, j, 512:768],
                                              in_=po[j][1][:, :256])
                    aT = pD.tile([128, 6, TPC], BF16)
                    for j in range(2):
                        _layernorm_rows(nc, sm, eps_t, ar[:, j, :])
                        for s in range(6):
                            pt2 = ps.tile([128, 512], F32, tag="t",
                                          name="pt2")
                            nc.tensor.transpose(
                                pt2[:, :128], ar[:, j, s * 128:(s + 1) * 128],
                                ident_f[:])
                            nc.vector.tensor_copy(
                                out=aT[:, s, j * 128:(j + 1) * 128],
                                in_=pt2[:, :128])

                    # ========== Phase E: FFN + LN + cls ==================
                    w1ch = pD.tile([128, 6, E], BF16)
                    nc.sync.dma_start(
                        out=w1ch[:],
                        in_=w1_d[:].rearrange("(s p) n -> p s n", p=128))
                    hT = pD.tile([128, 6, TPC], BF16)
                    for m in range(6):
                        ph = ps.tile([128, 512], F32, tag="t", name="ph")
                        for s in range(6):
                            nc.tensor.matmul(
                                ph[:, :TPC],
                                lhsT=w1ch[:, s, m * 128:(m + 1) * 128],
                                rhs=aT[:, s, :], start=(s == 0), stop=(s == 5))
                        nc.scalar.activation(
                            out=hT[:, m, :], in_=ph[:, :TPC], func=gelu_f,
                            bias=b1p_sb[:, m:m + 1], scale=1.0)
                    w2ch = pD.tile([128, 6, D], BF16)
                    nc.sync.dma_start(
                        out=w2ch[:],
                        in_=w2_d[:].rearrange("(s p) n -> p s n", p=128))
                    pe = [[psD.tile([128, 512], F32, tag=f"po{j}{n}",
                                    name=f"pe{j}{n}")
                           for n in range(2)] for j in range(2)]
                    er = pD.tile([128, 2, D], F32)
                    for j in range(2):
                        for s in range(6):
                            last = (s == 5) and ("b2" in skip)
                            nc.tensor.matmul(
                                pe[j][0][:],
                                lhsT=hT[:, s, j * 128:(j + 1) * 128],
                                rhs=w2ch[:, s, 0:512], start=(s == 0),
                                stop=last)
                            nc.tensor.matmul(
                                pe[j][1][:, :256],
                                lhsT=hT[:, s, j * 128:(j + 1) * 128],
                                rhs=w2ch[:, s, 512:768], start=(s == 0),
                                stop=last)
                        if "b2" not in skip:
                            nc.tensor.matmul(pe[j][0][:], lhsT=ones_b[:],
                                             rhs=b2_sb[:, 0:512],
                                             start=False, stop=True)
                            nc.tensor.matmul(pe[j][1][:, :256],
                                             lhsT=ones_b[:],
                                             rhs=b2_sb[:, 512:768],
                                             start=False, stop=True)
                        nc.vector.tensor_copy(out=er[:, j, 0:512],
                                              in_=pe[j][0][:])
                        nc.vector.tensor_copy(out=er[:, j, 512:768],
                                              in_=pe[j][1][:, :256])
                    for j in range(2):
                        _layernorm_rows(nc, sm, eps_t, er[:, j, :])
                        for s in range(6):
                            pt3 = ps.tile([128, 512], F32, tag="t",
                                          name="pt3")
                            nc.tensor.transpose(
                                pt3[:, :128], er[:, j, s * 128:(s + 1) * 128],
                                ident_f[:])
                            nc.vector.tensor_copy(
                                out=et[:, s, j * 128:(j + 1) * 128],
                                in_=pt3[:, :128])
                            if j == 0:
                                nc.scalar.activation(out=encT0[:, s:s + 1],
                                                     in_=pt3[:, 0:1],
                                                     func=AFT.Copy)
                    # cls head (f32 exact)
                    with tc.tile_pool(name="pscls", bufs=1,
                                      space="PSUM") as pscls:
                        pcl = pscls.tile([2, 512], F32)
                        for s in range(6):
                            nc.tensor.matmul(pcl[:, 0:1], lhsT=wc_sb[:, s, :],
                                             rhs=encT0[:, s:s + 1],
                                             start=(s == 0), stop=(s == 5))
                        cls_sb = sm.tile([2, 1], F32, tag="cls",
                                         name="cls_sb")
                        nc.scalar.activation(out=cls_sb[:], in_=pcl[:, 0:1],
                                             func=AFT.Identity, bias=bc_sb[:],
                                             scale=1.0)
                        nc.sync.dma_start(out=out_cls[:], in_=cls_sb[:])
            # mid pool (qT/ctxT) closed here

            # ============ Phase F: vocab projection (fully local) ========
            # per token-tile j: 64 chunks of 500 vocab cols; local logsumexp.
            NCH = 64
            CW = 500
            with tc.tile_pool(name="pF", bufs=1) as pF, \
                 tc.tile_pool(name="wsF", bufs=3) as wsF, \
                 tc.tile_pool(name="ltp", bufs=2) as ltp, \
                 tc.tile_pool(name="statp", bufs=2) as stp, \
                 tc.tile_pool(name="scrp", bufs=3) as scrp:
                bp_sb = None
                if "bp" not in skip:
                    bp_sb = pF.tile([1, V], BF16)
                    nc.sync.dma_start(out=bp_sb[:], in_=bp_d[:])
                lt = [ltp.tile([128, V], BF16, tag="lt", name=f"lt{j}")
                      for j in range(2)]
                nm = [stp.tile([128, NCH], F32, tag="nm", name=f"nm{j}")
                      for j in range(2)]
                se = [stp.tile([128, 16], F32, tag="se", name=f"se{j}")
                      for j in range(2)]
                for vc in range(NCH):
                    wpch = wsF.tile([128, 6, CW], BF16, tag="wpch",
                                    name="wpch")
                    nc.sync.dma_start(
                        out=wpch[:],
                        in_=wp_d[:, vc * CW:(vc + 1) * CW].rearrange(
                            "(s p) v -> p s v", p=128))
                    for j in range(2):
                        pl = ps.tile([128, 512], F32, tag="t", name="pl")
                        for s in range(6):
                            last = (s == 5) and ("bp" in skip)
                            nc.tensor.matmul(
                                pl[:, :CW],
                                lhsT=et[:, s, j * 128:(j + 1) * 128],
                                rhs=wpch[:, s, :],
                                start=(s == 0), stop=last)
                        if "bp" not in skip:
                            nc.tensor.matmul(
                                pl[:, :CW], lhsT=ones_b[:],
                                rhs=bp_sb[:, vc * CW:(vc + 1) * CW],
                                start=False, stop=True)
                        if (vc + j) % 2 == 0:
                            nc.vector.tensor_copy(
                                out=lt[j][:, vc * CW:(vc + 1) * CW],
                                in_=pl[:, :CW])
                        else:
                            nc.scalar.activation(
                                out=lt[j][:, vc * CW:(vc + 1) * CW],
                                in_=pl[:, :CW], func=AFT.Copy)
                        nc.vector.tensor_reduce(
                            out=nm[j][:, vc:vc + 1], in_=pl[:, :CW],
                            axis=AX.X, op=ALU.max, negate=True)
                for j in range(2):
                    gneg = stp.tile([128, 1], F32, tag="gneg", name=f"gn{j}")
                    nc.vector.tensor_reduce(out=gneg[:], in_=nm[j][:],
                                            axis=AX.X, op=ALU.min)
                    # sum of exp(x - M) per 2000-wide block, ACT accum
                    for bb in range(16):
                        scr = scrp.tile([128, 2000], BF16, tag="scr",
                                        name="scr")
                        nc.scalar.activation(
                            out=scr[:],
                            in_=lt[j][:, bb * 2000:(bb + 1) * 2000],
                            func=AFT.Exp, bias=gneg[:], scale=1.0,
                            accum_out=se[j][:, bb:bb + 1])
                    ssum = stp.tile([128, 1], F32, tag="ssum", name=f"ss{j}")
                    nc.vector.tensor_reduce(out=ssum[:], in_=se[j][:],
                                            axis=AX.X, op=ALU.add)
                    nc.scalar.activation(out=ssum[:], in_=ssum[:],
                                         func=AFT.Ln)
                    lse_neg = stp.tile([128, 1], F32, tag="lse",
                                       name=f"lse{j}")
                    nc.vector.tensor_sub(out=lse_neg[:], in0=gneg[:],
                                         in1=ssum[:])
                    for ob in range(16):
                        obuf = scrp.tile([128, 2000], F32, tag="obuf",
                                         name="obuf")
                        if ob % 2 == 0:
                            nc.scalar.activation(
                                out=obuf[:],
                                in_=lt[j][:, ob * 2000:(ob + 1) * 2000],
                                func=AFT.Identity, bias=lse_neg[:],
                                scale=1.0)
                        else:
                            nc.vector.tensor_scalar_add(
                                out=obuf[:],
                                in0=lt[j][:, ob * 2000:(ob + 1) * 2000],
                                scalar1=lse_neg[:])
                        nc.sync.dma_start(
                            out=out_logits[j * 128:(j + 1) * 128,
                                           ob * 2000:(ob + 1) * 2000],
                            in_=obuf[:])


# ======================= host side ==================================

def _host_prep(inputs):
    g = lambda k: np.asarray(inputs[k], dtype=np.float32)
    ids_full = np.asarray(inputs["input_tensor"]).astype(np.int32).reshape(-1)

    tok_emb = g("tok_emb")
    seg_emb = g("seg_emb")
    ln_eg, ln_eb = g("ln_emb_g"), g("ln_emb_b")
    Wq, bq = g("Wq"), g("bq")
    Wk, bk = g("Wk"), g("bk")
    Wv, bv = g("Wv"), g("bv")
    Wo, bo = g("Wo"), g("bo")
    ln_ag, ln_ab = g("ln_attn_g"), g("ln_attn_b")
    W1, b1 = g("W1"), g("b1")
    W2, b2 = g("W2"), g("b2")
    ln_fg, ln_fb = g("ln_ff_g"), g("ln_ff_b")
    Wp, bp = g("Wp"), g("bp")
    Wc, bc = g("Wc"), g("bc")

    # positional + segment "static" add, per sequence position
    p = np.arange(S, dtype=np.float32)[:, None]
    d2 = 2.0 * np.arange(D, dtype=np.float32) / D
    ang = p / np.power(np.float32(10000.0), d2)
    even = (np.arange(D) % 2) == 0
    pos = np.where(even[None, :], np.sin(ang), np.cos(ang)).astype(np.float32)
    seg_ids = np.where(np.arange(S) >= S // 2 + 1, 1, 0)
    static = (seg_emb[seg_ids] + pos).astype(np.float32)        # [S, D]
    static_full = np.tile(static, (B, 1))                        # [NT, D]

    scale = 1.0 / np.sqrt(np.float32(S))
    Wq_t = Wq.transpose(1, 0, 2).reshape(D, H * E)
    Wk_t = Wk.transpose(1, 0, 2).reshape(D, H * E)
    Wv_t = Wv.transpose(1, 0, 2).reshape(D, H * E)
    wq_f = (ln_eg[:, None] * Wq_t) * scale
    wk_f = ln_eg[:, None] * Wk_t
    wv_f = ln_eg[:, None] * Wv_t
    bq_f = (ln_eb @ Wq_t + bq.reshape(-1)) * scale
    bk_f = ln_eb @ Wk_t + bk.reshape(-1)
    bv_f = ln_eb @ Wv_t + bv.reshape(-1)
    wqkv = np.concatenate([wq_f, wk_f, wv_f], axis=1).astype(BF)
    bqk = np.concatenate([bq_f, bk_f]).reshape(144, 128).T
    bqk = np.ascontiguousarray(bqk, dtype=np.float32)
    bv_r = bv_f.reshape(1, H * E).astype(BF)

    wo_b = Wo.astype(BF)
    bo_r = bo.reshape(1, D).astype(BF)
    w1_f = (ln_ag[:, None] * W1).astype(BF)
    b1_f = (ln_ab @ W1 + b1).astype(np.float32)
    b1p = np.ascontiguousarray(b1_f.reshape(6, 128).T, dtype=np.float32)
    w2_b = W2.astype(BF)
    b2_r = b2.reshape(1, D).astype(BF)
    wp_f = (ln_fg[:, None] * Wp).astype(BF)
    bp_f = (ln_fb @ Wp + bp).astype(np.float32)
    wc_f = np.ascontiguousarray(ln_fg[:, None] * Wc, dtype=np.float32)
    bc_f = np.ascontiguousarray((ln_fb @ Wc + bc).reshape(2, 1),
                                dtype=np.float32)

    skip = set()
    if not np.any(np.concatenate([bq_f, bk_f])):
        skip.add("bqk")
    if not np.any(bv_f):
        skip.add("bv")
    if not np.any(bo):
        skip.add("bo")
    if not np.any(b2):
        skip.add("b2")
    if not np.any(bp_f):
        skip.add("bp")

    tok_emb_c = np.ascontiguousarray(tok_emb, dtype=np.float32)
    bp_r = np.ascontiguousarray(bp_f.reshape(1, V).astype(BF))
    in_maps = []
    for c in range(NC):
        own = np.arange(c * TPC, (c + 1) * TPC)
        pr = c ^ 1
        pair = np.arange(pr * TPC, (pr + 1) * TPC)
        toks = np.concatenate([own, pair])
        in_maps.append(dict(
            ids=np.ascontiguousarray(
                ids_full[toks].reshape(4, 128, 1), dtype=np.int32),
            static=np.ascontiguousarray(
                static_full[toks].reshape(4, 128, D), dtype=np.float32),
            tok_emb=tok_emb_c,
            wqkv=wqkv, bqk=bqk, bv=bv_r,
            wo=wo_b, bo=bo_r,
            w1=w1_f, b1p=b1p, w2=w2_b, b2=b2_r,
            wp=wp_f, bp=bp_r,
            wc=wc_f, bc=bc_f,
        ))
    return in_maps, frozenset(skip)


_NC_CACHE = {}


def _get_nc(sim_gelu=False, skip=frozenset()):
    key = (sim_gelu, skip)
    if key not in _NC_CACHE:
        _NC_CACHE[key] = build_nc(sim_gelu=sim_gelu, skip=skip)
    return _NC_CACHE[key]


def kernel(**inputs):
    in_maps, skip = _host_prep(inputs)
    nc = _get_nc(sim_gelu=False, skip=skip)
    res = run_bass_kernel_spmd(nc, in_maps, list(range(NC)))
    return _assemble(res.results)


def _assemble(results):
    token_pred = np.concatenate(
        [results[c]["out_logits"] for c in range(NC)], axis=0)
    token_pred = np.ascontiguousarray(
        token_pred.reshape(B, S, V), dtype=np.float32)
    cls = np.stack([results[2 * b]["out_cls"][:, 0] for b in range(B)])
    return token_pred, np.ascontiguousarray(cls, dtype=np.float32)
